# revision 1
# baseline (speedup 1.0000x reference)
"""Bass/Trainium2 kernel for nn_CWRRTESWindowCell (scatter_memory).

Sharding: data-parallel over batch across 8 NeuronCores (B=64 -> 8/core);
the augmented engram table and small params are replicated.

Host prep (param folding + index prep only):
  - fold gate=sigmoid(gate_logit), temp=softplus(temp)+0.3 and sal_W into an
    augmented table  aug[(m,h)] = [engram[m,h,:]*gate[h,:] (128 f32) |
    per-head salience-logit contribution (4 f32)]  so one gathered row
    carries both the value and its logit term,
  - uint32 rolling-hash n-gram lookup indices (as in the reference),
  - EWb[token] (embed@sal_W+b)/temp dense per position (128-row lookup).

Device (per core), for each (batch b, 128-token tile):
  - 4 indirect-DMA gathers (one per head): 128 rows x 528B from aug,
  - logits = EWb + sum of the 4 gathered logit columns; exps = exp(l)*mask
    (no max-subtraction: |logits| << 1 by construction),
  - PE accumulates over the 16 tiles of b in PSUM:
      accT[d, h] += gathered_h^T @ exps[:, h]   (weighted engram sum)
      ws[v, h]   += onehot(tok)^T @ exps        (per-vocab weight sums)
  - finalize b: accT += embed-part via ws, S = colsum(ws), PE-transpose,
    divide by S+1e-6, RMSNorm, sigmoid gate head, store [8, 1024] shard.

Measured: ~749 us on 8 trn2 cores, rel err 4.6e-6. The kernel is 99.9%
packed on the gpsimd indirect-DMA chain: 512 gather ops x ~1.46 us
(~1.1 us Q7 descriptor-gen busy + ~0.36 us sequencer dispatch); all other
engines (PE 41%, DVE 25%) hide fully beneath it. Hardware consumes exactly
one index per partition per indirect DMA (CoreSim's multi-index batching
does not exist on HW). The cheap-descriptor alternative (dma_gather +
host-side dedup of each (core,head)'s <=16384-row working set into int16
compact tables, sim-verified, ~32 ops/core) crashes NRT in this axon
runtime - first thing to retry if ext-isa ucode becomes loadable.
"""
import sys

sys.path.insert(0, "/opt/trn_rl_repo")

import numpy as np

# ---- problem constants (hardcoded per contest contract) ----
B, T, O, D, V = 64, 2048, 3, 512, 128
M, NG, H, HD = 100000, 4, 4, 128
NCORES = 8
BL = B // NCORES          # 8 batches per core
P = 128                   # partition / token-tile size
NT = T // P               # 16 token tiles per batch
ELEM = HD + 4             # 132 floats per augmented row
GT = 2                    # token tiles gathered per indirect DMA call
EPS_RMS = 1e-6


def _engram_primes():
    ps = []
    base = 131
    for h in range(H):
        x = base + h * 1009
        row = []
        for _ in range(NG):
            row.append(x)
            x = x * 31 + 1
        ps.append(row)
    return np.array(ps, dtype=np.uint32)


_NC_CACHE = {}


def _build_nc():
    if "nc" in _NC_CACHE:
        return _NC_CACHE["nc"]
    import concourse.bass as bass
    import concourse.tile as tile
    from concourse import bacc, mybir

    f32 = mybir.dt.float32
    i32 = mybir.dt.int32
    Alu = mybir.AluOpType
    Act = mybir.ActivationFunctionType
    X = mybir.AxisListType.X

    nc = bacc.Bacc(None, target_bir_lowering=False)

    aug = nc.declare_dram_parameter("aug", [M * H, ELEM], f32, isOutput=False)
    emb = nc.declare_dram_parameter("emb", [V, D], f32, isOutput=False)
    ident = nc.declare_dram_parameter("ident", [P, P], f32, isOutput=False)
    iotaf = nc.declare_dram_parameter("iotaf", [P, P], f32, isOutput=False)
    gwr = nc.declare_dram_parameter("gwr", [4, HD], f32, isOutput=False)
    rmsr = nc.declare_dram_parameter("rmsr", [4, HD], f32, isOutput=False)
    gb4 = nc.declare_dram_parameter("gb4", [4, 1], f32, isOutput=False)
    onesc = nc.declare_dram_parameter("onesc", [P, 1], f32, isOutput=False)
    ones4c = nc.declare_dram_parameter("ones4c", [4, 1], f32, isOutput=False)
    ones4r = nc.declare_dram_parameter("ones4r", [1, 4], f32, isOutput=False)
    ewt = nc.declare_dram_parameter("ewt", [P, BL * NT * 4], f32, isOutput=False)
    idx4 = nc.declare_dram_parameter("idx4", [P, BL * NT * 4], i32, isOutput=False)
    tokf = nc.declare_dram_parameter("tokf", [P, BL * NT], f32, isOutput=False)
    maskf = nc.declare_dram_parameter("maskf", [P, BL * NT], f32, isOutput=False)
    out_d = nc.declare_dram_parameter("out", [BL, 8, HD], f32, isOutput=True)

    with tile.TileContext(nc) as tc:
        with tc.tile_pool(name="const", bufs=1) as cp, \
             tc.tile_pool(name="work", bufs=10) as wp, \
             tc.tile_pool(name="small", bufs=4) as sp, \
             tc.tile_pool(name="fin", bufs=1) as fp, \
             tc.tile_pool(name="accp", bufs=2, space="PSUM") as accp, \
             tc.tile_pool(name="wsp", bufs=3, space="PSUM") as wsp, \
             tc.tile_pool(name="tinyp", bufs=3, space="PSUM") as tinyp:

            # ---- constant loads ----
            emb_t = cp.tile([V, D], f32, tag="emb")
            nc.sync.dma_start(out=emb_t[:], in_=emb[:, :])
            iota_t = cp.tile([P, P], f32, tag="iota")
            nc.sync.dma_start(out=iota_t[:], in_=iotaf[:, :])
            gwr_t = cp.tile([4, HD], f32, tag="gwr")
            nc.sync.dma_start(out=gwr_t[:], in_=gwr[:, :])
            rmsr_t = cp.tile([4, HD], f32, tag="rmsr")
            nc.sync.dma_start(out=rmsr_t[:], in_=rmsr[:, :])
            gb4_t = cp.tile([4, 1], f32, tag="gb4")
            nc.sync.dma_start(out=gb4_t[:], in_=gb4[:, :])
            onesc_t = cp.tile([P, 1], f32, tag="onesc")
            nc.sync.dma_start(out=onesc_t[:], in_=onesc[:, :])
            ones4c_t = cp.tile([4, 1], f32, tag="ones4c")
            nc.sync.dma_start(out=ones4c_t[:], in_=ones4c[:, :])
            ones4r_t = cp.tile([1, 4], f32, tag="ones4r")
            nc.sync.dma_start(out=ones4r_t[:], in_=ones4r[:, :])
            ident_t = cp.tile([P, P], f32, tag="ident")
            nc.sync.dma_start(out=ident_t[:], in_=ident[:, :])
            ewt_t = cp.tile([P, BL * NT * 4], f32, tag="ewt")
            nc.sync.dma_start(out=ewt_t[:], in_=ewt[:, :])
            idx4_t = cp.tile([P, BL * NT * 4], i32, tag="idx4")
            nc.sync.dma_start(out=idx4_t[:], in_=idx4[:, :])
            tokf_t = cp.tile([P, BL * NT], f32, tag="tokf")
            nc.sync.dma_start(out=tokf_t[:], in_=tokf[:, :])
            maskf_t = cp.tile([P, BL * NT], f32, tag="maskf")
            nc.sync.dma_start(out=maskf_t[:], in_=maskf[:, :])

            for b in range(BL):
                accT = accp.tile([P, 4], f32, tag="accT")
                ws = wsp.tile([P, 4], f32, tag="ws")
                for tile_i in range(NT):
                    c = b * NT + tile_i
                    first = tile_i == 0
                    gs = []
                    for h in range(4):
                        gh = wp.tile([P, ELEM], f32, tag=f"g{h}")
                        nc.gpsimd.indirect_dma_start(
                            out=gh[:],
                            out_offset=None,
                            in_=aug[:, :],
                            in_offset=bass.IndirectOffsetOnAxis(
                                ap=idx4_t[:, c * 4 + h:c * 4 + h + 1], axis=0
                            ),
                        )
                        gs.append(gh)
                    # logits = EWb[tok] + sum_h logit cols
                    l = wp.tile([P, 4], f32, tag="l")
                    nc.vector.tensor_tensor(
                        out=l[:],
                        in0=ewt_t[:, c * 4:(c + 1) * 4],
                        in1=gs[0][:, HD:HD + 4],
                        op=Alu.add,
                    )
                    for h in range(1, 4):
                        nc.vector.tensor_tensor(
                            out=l[:], in0=l[:], in1=gs[h][:, HD:HD + 4], op=Alu.add,
                        )
                    e_raw = wp.tile([P, 4], f32, tag="e_raw")
                    nc.scalar.activation(out=e_raw[:], in_=l[:], func=Act.Exp)
                    e = wp.tile([P, 4], f32, tag="e")
                    nc.vector.tensor_scalar(
                        out=e[:], in0=e_raw[:],
                        scalar1=maskf_t[:, c:c + 1], scalar2=None,
                        op0=Alu.mult,
                    )
                    # onehot_T[t, v] = (iota[v] == tok[t])
                    oh = wp.tile([P, P], f32, tag="oh")
                    nc.vector.tensor_scalar(
                        out=oh[:], in0=iota_t[:],
                        scalar1=tokf_t[:, c:c + 1], scalar2=None,
                        op0=Alu.is_equal,
                    )
                    nc.tensor.matmul(
                        out=ws[:], lhsT=oh[:], rhs=e[:],
                        start=first, stop=(tile_i == NT - 1),
                    )
                    # accT[:, h] += g_h^T @ e[:, h]  (stationary = gathered tile)
                    for h in range(4):
                        nc.tensor.matmul(
                            out=accT[:, h:h + 1],
                            lhsT=gs[h][:, :HD],
                            rhs=e[:, h:h + 1],
                            start=(first and h == 0), stop=False,
                        )
                # ---- finalize batch b ----
                ws_sb = sp.tile([P, 4], f32, tag="ws_sb")
                nc.vector.tensor_copy(out=ws_sb[:], in_=ws[:])
                # accT[:, h] += emb_h^T @ ws[:, h]
                for h in range(4):
                    nc.tensor.matmul(
                        out=accT[:, h:h + 1],
                        lhsT=emb_t[:, h * HD:(h + 1) * HD],
                        rhs=ws_sb[:, h:h + 1],
                        start=False, stop=(h == 3),
                    )
                # S[h] = sum_v ws[v, h]   -> [4, 1]
                s_p = tinyp.tile([4, 1], f32, tag="tiny")
                nc.tensor.matmul(
                    out=s_p[:], lhsT=ws_sb[:], rhs=onesc_t[:],
                    start=True, stop=True,
                )
                s_sb = sp.tile([4, 1], f32, tag="s_sb")
                nc.vector.tensor_copy(out=s_sb[:], in_=s_p[:])
                # transpose accT [128, 4] -> wvT [4, 128]
                accT_sb = sp.tile([P, 4], f32, tag="accT_sb")
                nc.vector.tensor_copy(out=accT_sb[:], in_=accT[:])
                wvT_p = tinyp.tile([4, P], f32, tag="tiny")
                nc.tensor.transpose(out=wvT_p[:], in_=accT_sb[:], identity=ident_t[:])
                # wv = wvT / (S + 1e-6)
                seps = sp.tile([4, 1], f32, tag="seps")
                nc.vector.tensor_scalar(
                    out=seps[:], in0=s_sb[:], scalar1=1e-6, scalar2=None, op0=Alu.add,
                )
                rec = sp.tile([4, 1], f32, tag="rec")
                nc.vector.reciprocal(out=rec[:], in_=seps[:])
                wv = sp.tile([4, HD], f32, tag="wv")
                nc.vector.tensor_scalar(
                    out=wv[:], in0=wvT_p[:], scalar1=rec[:, :1], scalar2=None,
                    op0=Alu.mult,
                )
                # RMS over all 512 = 4 partitions x 128
                sq = sp.tile([4, HD], f32, tag="sq")
                nc.vector.tensor_tensor(out=sq[:], in0=wv[:], in1=wv[:], op=Alu.mult)
                sqs = sp.tile([4, 1], f32, tag="sqs")
                nc.vector.tensor_reduce(out=sqs[:], in_=sq[:], axis=X, op=Alu.add)
                rmsp = tinyp.tile([4, 1], f32, tag="tiny")
                nc.tensor.matmul(
                    out=rmsp[0:1, 0:1], lhsT=sqs[:], rhs=ones4c_t[:],
                    start=True, stop=True,
                )
                msq = sp.tile([1, 1], f32, tag="msq")
                nc.vector.tensor_scalar(
                    out=msq[:], in0=rmsp[0:1, 0:1], scalar1=1.0 / D, scalar2=EPS_RMS,
                    op0=Alu.mult, op1=Alu.add,
                )
                sqr = sp.tile([1, 1], f32, tag="sqr")
                nc.scalar.activation(out=sqr[:], in_=msq[:], func=Act.Sqrt)
                rinv = sp.tile([1, 1], f32, tag="rinv")
                nc.vector.reciprocal(out=rinv[:], in_=sqr[:])
                r4p = tinyp.tile([4, 1], f32, tag="tiny")
                nc.tensor.matmul(
                    out=r4p[:], lhsT=ones4r_t[:], rhs=rinv[:], start=True, stop=True
                )
                r4_sb = sp.tile([4, 1], f32, tag="r4_sb")
                nc.vector.tensor_copy(out=r4_sb[:], in_=r4p[:])
                wvn = sp.tile([4, HD], f32, tag="wvn")
                nc.vector.tensor_scalar(
                    out=wvn[:], in0=wv[:], scalar1=r4_sb[:, :1], scalar2=None,
                    op0=Alu.mult,
                )
                wvf = sp.tile([4, HD], f32, tag="wvf")
                nc.vector.tensor_tensor(out=wvf[:], in0=wvn[:], in1=rmsr_t[:], op=Alu.mult)
                # gate head: u = sigmoid(wv . gate_W + gate_b) * (S > 0)
                gwm = sp.tile([4, HD], f32, tag="gwm")
                nc.vector.tensor_tensor(out=gwm[:], in0=wv[:], in1=gwr_t[:], op=Alu.mult)
                gl = sp.tile([4, 1], f32, tag="gl")
                nc.vector.tensor_reduce(out=gl[:], in_=gwm[:], axis=X, op=Alu.add)
                glb = sp.tile([4, 1], f32, tag="glb")
                nc.vector.tensor_tensor(out=glb[:], in0=gl[:], in1=gb4_t[:], op=Alu.add)
                sg = sp.tile([4, 1], f32, tag="sg")
                nc.scalar.activation(out=sg[:], in_=glb[:], func=Act.Sigmoid)
                valid = sp.tile([4, 1], f32, tag="valid")
                nc.vector.tensor_scalar(
                    out=valid[:], in0=s_sb[:], scalar1=0.0, scalar2=None, op0=Alu.is_gt,
                )
                u = sp.tile([4, 1], f32, tag="u")
                nc.vector.tensor_tensor(out=u[:], in0=sg[:], in1=valid[:], op=Alu.mult)
                ue = sp.tile([4, HD], f32, tag="ue")
                nc.vector.tensor_scalar(
                    out=ue[:], in0=wvf[:], scalar1=0.0, scalar2=u[:, :1],
                    op0=Alu.mult, op1=Alu.add,
                )
                nc.sync.dma_start(out=out_d[b, 0:4, :], in_=wvf[:])
                nc.sync.dma_start(out=out_d[b, 4:8, :], in_=ue[:])

    nc.finalize()
    _NC_CACHE["nc"] = nc
    return nc


def _host_prep(inputs):
    tokens_w = np.asarray(inputs["tokens_w"], dtype=np.int32)
    prev_ids = np.asarray(inputs["prev_ids_overlap"], dtype=np.int32)
    mask_bool = np.asarray(inputs["mask_bool"])
    embed_table = np.asarray(inputs["embed_table"], dtype=np.float32)
    engram_table = np.asarray(inputs["engram_table"], dtype=np.float32)
    gate_logit = np.asarray(inputs["gate_logit"], dtype=np.float32)
    temp = np.asarray(inputs["temp"], dtype=np.float32)
    sal_W = np.asarray(inputs["sal_W"], dtype=np.float32)
    sal_b = np.asarray(inputs["sal_b"], dtype=np.float32)
    gate_W = np.asarray(inputs["gate_W"], dtype=np.float32)
    gate_b = np.asarray(inputs["gate_b"], dtype=np.float32)
    rms_scale = np.asarray(inputs["rms_scale"], dtype=np.float32)

    # ---- hashed n-gram lookup (uint32 rolling hash, as in reference) ----
    cur = np.where(tokens_w == 0, 0, tokens_w)
    prv = np.where(prev_ids == 0, 0, prev_ids)
    full_seq = np.concatenate([prv, cur], axis=1).astype(np.uint32)  # (B, O+T)
    primes = _engram_primes()                                        # (H, NG)
    hash_sums = np.zeros((B, T, H), dtype=np.uint32)
    for i in range(NG):
        chunk = full_seq[:, O - i:O + T - i]                         # (B, T)
        hash_sums += chunk[:, :, None] * primes[None, None, :, i]
    lookup = (hash_sums % np.uint32(M)).astype(np.int64)             # (B, T, H)
    rows = (lookup * H + np.arange(H)[None, None, :]).astype(np.int32)

    # ---- param folding ----
    gate = 1.0 / (1.0 + np.exp(-gate_logit.astype(np.float64)))      # (H, HD)
    tf = np.log1p(np.exp(temp.astype(np.float64))) + 0.3             # (H,)
    gate32 = gate.astype(np.float32)
    gated = engram_table * gate32[None, :, :]                        # (M, H, HD)
    SWt = (sal_W.astype(np.float64) / tf[None, :]).astype(np.float32)  # (D, 4)
    SWt_r = SWt.reshape(H, HD, 4)
    aug = np.empty((M * H, ELEM), dtype=np.float32)
    aug[:, :HD] = gated.reshape(M * H, HD)
    for h in range(H):
        aug[h::H, HD:] = gated[:, h, :] @ SWt_r[h]
    EWb = ((embed_table.astype(np.float64) @ sal_W.astype(np.float64)
            + sal_b[None, :].astype(np.float64)) / tf[None, :]).astype(np.float32)

    # ---- per-core input layout: [p, (b, tile)] with p = t % 128 ----
    def to_pt(x2d, dtype):  # (B, T) -> (P, B*NT)
        return np.ascontiguousarray(
            x2d.reshape(B, NT, P).transpose(2, 0, 1).reshape(P, B * NT)
        ).astype(dtype)

    tok_pt_f = to_pt(tokens_w, np.float32)
    mask_pt = to_pt(mask_bool.astype(np.float32), np.float32)
    idx_pt = np.ascontiguousarray(
        rows.reshape(B, NT, P, H).transpose(2, 0, 1, 3).reshape(P, B * NT * H)
    )

    # EW logits per position: EWb[token], in the same [p, (b,tile,h')] layout
    EW_full = EWb[tokens_w]                                          # (B, T, 4)
    ew_pt = np.ascontiguousarray(
        EW_full.reshape(B, NT, P, 4).transpose(2, 0, 1, 3).reshape(P, B * NT * 4)
    ).astype(np.float32)

    iota_f = np.ascontiguousarray(
        np.broadcast_to(np.arange(P, dtype=np.float32), (P, P))
    )
    gwr = np.ascontiguousarray(
        np.broadcast_to(gate_W[:, 0][None, :], (4, HD))
    ).astype(np.float32)
    rmsr = rms_scale.reshape(4, HD).copy()
    gb4 = np.full((4, 1), float(gate_b[0]), dtype=np.float32)

    shared = {
        "aug": aug, "emb": embed_table, "iotaf": iota_f,
        "ident": np.eye(P, dtype=np.float32),
        "gwr": gwr, "rmsr": rmsr, "gb4": gb4,
        "onesc": np.ones((P, 1), dtype=np.float32),
        "ones4c": np.ones((4, 1), dtype=np.float32),
        "ones4r": np.ones((1, 4), dtype=np.float32),
    }
    in_maps = []
    for k in range(NCORES):
        cs, ce = k * BL * NT, (k + 1) * BL * NT
        m = dict(shared)
        m["idx4"] = np.ascontiguousarray(idx_pt[:, cs * 4:ce * 4])
        m["ewt"] = np.ascontiguousarray(ew_pt[:, cs * 4:ce * 4])
        m["tokf"] = np.ascontiguousarray(tok_pt_f[:, cs:ce])
        m["maskf"] = np.ascontiguousarray(mask_pt[:, cs:ce])
        in_maps.append(m)
    return in_maps


def _run(inputs, trace=False, **kw):
    from concourse.bass_utils import run_bass_kernel_spmd

    nc = _build_nc()
    in_maps = _host_prep(inputs)
    r = run_bass_kernel_spmd(
        nc, in_maps, list(range(NCORES)), trace=trace, **kw
    )
    out = np.concatenate([r.results[k]["out"].reshape(BL, 2 * D)
                          for k in range(NCORES)], axis=0)
    return out, r


def kernel(**inputs):
    out, _ = _run(inputs, trace=False)
    return out



# revision 8
# speedup vs baseline: 1.7370x; 1.7370x over previous
"""Bass/Trainium2 kernel for nn_CWRRTESWindowCell (scatter_memory).

Sharding: data-parallel over batch across 8 NeuronCores (B=64 -> 8/core);
the augmented engram table and small params are replicated.

Host prep (param folding + index prep only):
  - fold gate=sigmoid(gate_logit), temp=softplus(temp)+0.3 and sal_W into an
    augmented table  aug[(m,h)] = [engram[m,h,:]*gate[h,:] (128 f32) |
    per-head salience-logit contribution (4 f32)],
  - uint32 rolling-hash n-gram lookup indices (as in the reference),
  - MASK COMPACTION: only tokens with mask=1 contribute anything (their
    softmax weight is exactly 0 otherwise), so each batch's valid tokens
    (~1024 of 2048) are packed into NTC=9 tiles of 128; padding slots get
    EW logit -30 (=> weight exp(-30)~1e-13, numerically nil) and index 0.
    This halves the indirect-DMA gather count, which is the bottleneck:
    each 128-row gather costs ~1.4us of Pool-engine SWDGE descriptor
    generation (994ns fixed + 0.34ns/desc + ~310ns dispatch gap), fully
    serialized - measured on HW, matching CoreSim's cost model. (Probed:
    multi-index offset APs do NOT batch on HW; one index per partition,
    extra out columns just read contiguantly past the indexed row.)

Device (per core), for each (batch b, compacted 128-token tile):
  - 4 indirect-DMA gathers (one per head): 128 rows x 528B from aug,
  - logits l = EWb + sum of the 4 gathered logit columns; e = exp(l)
    (no max-subtraction: |l| << 1; mask folded into EWb),
  - PE with e as the STATIONARY operand (tiny 128x4 weight load instead of
    five 128x128 loads per tile in the old layout):
      acc2[h, h'*128:...] += e^T @ g_{h'}[:, :128]   (only diag blocks used)
      wsT[h, v] += e^T @ onehot(tok)                 (per-vocab weight sums)
      accS[h, 0] += e^T @ ones                       (softmax denominators)
  - finalize b: wsT -> transpose -> ws128; acc2 += ws128^T-stationary
    emb matmul (adds the embed part of x); extract diag blocks, divide by
    S+1e-6, RMSNorm (Rsqrt), sigmoid gate via the Exp table, store shard.

Measured: 754us baseline -> this version targets ~410us (288 gathers/core).
"""
import sys

sys.path.insert(0, "/opt/trn_rl_repo")

import numpy as np

# ---- problem constants (hardcoded per contest contract) ----
B, T, O, D, V = 64, 2048, 3, 512, 128
M, NG, H, HD = 100000, 4, 4, 128
NCORES = 8
BL = B // NCORES          # 8 batches per core
P = 128                   # partition / token-tile size
NTC = 9                   # compacted token tiles per batch (max valid 1080)
CP = NTC * P              # compacted slots per batch
ELEM = HD + 4             # 132 floats per augmented row
EPS_RMS = 1e-6
PAD_LOGIT = -30.0


def _engram_primes():
    ps = []
    base = 131
    for h in range(H):
        x = base + h * 1009
        row = []
        for _ in range(NG):
            row.append(x)
            x = x * 31 + 1
        ps.append(row)
    return np.array(ps, dtype=np.uint32)


_NC_CACHE = {}


def _build_nc():
    if "nc" in _NC_CACHE:
        return _NC_CACHE["nc"]
    import concourse.bass as bass
    import concourse.tile as tile
    from concourse import bacc, mybir

    f32 = mybir.dt.float32
    i32 = mybir.dt.int32
    Alu = mybir.AluOpType
    Act = mybir.ActivationFunctionType
    X = mybir.AxisListType.X

    nc = bacc.Bacc(None, target_bir_lowering=False)

    aug = nc.declare_dram_parameter("aug", [M * H, ELEM], f32, isOutput=False)
    emb = nc.declare_dram_parameter("emb", [V, D], f32, isOutput=False)
    iotaf = nc.declare_dram_parameter("iotaf", [P, P], f32, isOutput=False)
    ident4 = nc.declare_dram_parameter("ident4", [4, 4], f32, isOutput=False)
    gwr = nc.declare_dram_parameter("gwr", [4, HD], f32, isOutput=False)
    rmsr = nc.declare_dram_parameter("rmsr", [4, HD], f32, isOutput=False)
    ngb4 = nc.declare_dram_parameter("ngb4", [4, 1], f32, isOutput=False)
    onesc = nc.declare_dram_parameter("onesc", [P, 1], f32, isOutput=False)
    ones4c = nc.declare_dram_parameter("ones4c", [4, 1], f32, isOutput=False)
    ones4r = nc.declare_dram_parameter("ones4r", [1, 4], f32, isOutput=False)
    ewt = nc.declare_dram_parameter("ewt", [P, BL * NTC * 4], f32, isOutput=False)
    idx4 = nc.declare_dram_parameter("idx4", [P, BL * NTC * 4], i32, isOutput=False)
    tokf = nc.declare_dram_parameter("tokf", [P, BL * NTC], f32, isOutput=False)
    out_d = nc.declare_dram_parameter("out", [BL, 8, HD], f32, isOutput=True)

    with tile.TileContext(nc) as tc:
        with tc.tile_pool(name="const", bufs=1) as cp, \
             tc.tile_pool(name="work", bufs=12) as wp, \
             tc.tile_pool(name="small", bufs=8) as sp, \
             tc.tile_pool(name="fin", bufs=2) as fp, \
             tc.tile_pool(name="accp", bufs=2, space="PSUM") as accp, \
             tc.tile_pool(name="wsp", bufs=2, space="PSUM") as wsp, \
             tc.tile_pool(name="tinyp", bufs=1, space="PSUM") as tinyp:

            # ---- constant loads (gather-critical idx4 first) ----
            idx4_t = cp.tile([P, BL * NTC * 4], i32, tag="idx4")
            nc.sync.dma_start(out=idx4_t[:], in_=idx4[:, :])
            ewt_t = cp.tile([P, BL * NTC * 4], f32, tag="ewt")
            nc.sync.dma_start(out=ewt_t[:], in_=ewt[:, :])
            tokf_t = cp.tile([P, BL * NTC], f32, tag="tokf")
            nc.sync.dma_start(out=tokf_t[:], in_=tokf[:, :])
            iota_t = cp.tile([P, P], f32, tag="iota")
            nc.sync.dma_start(out=iota_t[:], in_=iotaf[:, :])
            onesc_t = cp.tile([P, 1], f32, tag="onesc")
            nc.sync.dma_start(out=onesc_t[:], in_=onesc[:, :])
            emb_t = cp.tile([V, D], f32, tag="emb")
            nc.sync.dma_start(out=emb_t[:], in_=emb[:, :])
            ident4_t = cp.tile([4, 4], f32, tag="ident4")
            nc.sync.dma_start(out=ident4_t[:], in_=ident4[:, :])
            gwr_t = cp.tile([4, HD], f32, tag="gwr")
            nc.sync.dma_start(out=gwr_t[:], in_=gwr[:, :])
            rmsr_t = cp.tile([4, HD], f32, tag="rmsr")
            nc.sync.dma_start(out=rmsr_t[:], in_=rmsr[:, :])
            ngb4_t = cp.tile([4, 1], f32, tag="ngb4")
            nc.sync.dma_start(out=ngb4_t[:], in_=ngb4[:, :])
            ones4c_t = cp.tile([4, 1], f32, tag="ones4c")
            nc.sync.dma_start(out=ones4c_t[:], in_=ones4c[:, :])
            ones4r_t = cp.tile([1, 4], f32, tag="ones4r")
            nc.sync.dma_start(out=ones4r_t[:], in_=ones4r[:, :])

            for b in range(BL):
                acc2 = accp.tile([4, D], f32, tag="acc2")
                wsT = wsp.tile([4, P], f32, tag="wsT")
                accS = tinyp.tile([4, 1], f32, tag="accS")
                for j in range(NTC):
                    c = b * NTC + j
                    first = j == 0
                    last = j == NTC - 1
                    gs = []
                    for h in range(4):
                        gh = wp.tile([P, ELEM], f32, tag=f"g{h}")
                        nc.gpsimd.indirect_dma_start(
                            out=gh[:],
                            out_offset=None,
                            in_=aug[:, :],
                            in_offset=bass.IndirectOffsetOnAxis(
                                ap=idx4_t[:, c * 4 + h:c * 4 + h + 1], axis=0
                            ),
                        )
                        gs.append(gh)
                    # logits = EWb[tok] + sum of the 4 gathered logit cols
                    l = sp.tile([P, 4], f32, tag="l")
                    nc.vector.tensor_tensor(
                        out=l[:],
                        in0=ewt_t[:, c * 4:(c + 1) * 4],
                        in1=gs[0][:, HD:HD + 4],
                        op=Alu.add,
                    )
                    for h in range(1, 4):
                        nc.vector.tensor_tensor(
                            out=l[:], in0=l[:], in1=gs[h][:, HD:HD + 4], op=Alu.add,
                        )
                    e = sp.tile([P, 4], f32, tag="e")
                    nc.scalar.activation(out=e[:], in_=l[:], func=Act.Exp)
                    # onehot[t, v] = (iota[v] == tok[t])
                    oh = sp.tile([P, P], f32, tag="oh")
                    nc.vector.tensor_scalar(
                        out=oh[:], in0=iota_t[:],
                        scalar1=tokf_t[:, c:c + 1], scalar2=None,
                        op0=Alu.is_equal,
                    )
                    # PE: e is stationary [128, 4] for all three products
                    for h in range(4):
                        nc.tensor.matmul(
                            out=acc2[:, h * HD:(h + 1) * HD],
                            lhsT=e[:], rhs=gs[h][:, :HD],
                            start=first, stop=False,
                        )
                    nc.tensor.matmul(
                        out=wsT[:], lhsT=e[:], rhs=oh[:],
                        start=first, stop=last,
                    )
                    nc.tensor.matmul(
                        out=accS[:], lhsT=e[:], rhs=onesc_t[:],
                        start=first, stop=last,
                    )
                # ---- finalize batch b ----
                wsT_sb = fp.tile([4, P], f32, tag="wsT_sb")
                nc.vector.tensor_copy(out=wsT_sb[:], in_=wsT[:])
                ws128_p = tinyp.tile([P, 4], f32, tag="ws128")
                nc.tensor.matmul(
                    out=ws128_p[:], lhsT=wsT_sb[:], rhs=ident4_t[:],
                    start=True, stop=True,
                )
                ws128_sb = fp.tile([P, 4], f32, tag="ws128_sb")
                nc.vector.tensor_copy(out=ws128_sb[:], in_=ws128_p[:])
                # acc2[h, :] += sum_v ws[v, h] * emb[v, :]  (diag blocks used)
                nc.tensor.matmul(
                    out=acc2[:], lhsT=ws128_sb[:], rhs=emb_t[:],
                    start=False, stop=True,
                )
                s_sb = fp.tile([4, 1], f32, tag="s_sb")
                nc.vector.tensor_copy(out=s_sb[:], in_=accS[:])
                # wv[h, :] = acc2[h, h*HD:(h+1)*HD] / (S + 1e-6)
                # (PSUM reads must start at partition 0: copy whole tile to
                #  SBUF first, then slice per-partition there)
                acc2_sb = fp.tile([4, D], f32, tag="acc2_sb")
                nc.vector.tensor_copy(out=acc2_sb[:], in_=acc2[:])
                # engines can't start an AP at partition h>0, but DMA APs are
                # byte-addressed: extract diag blocks with SBUF->SBUF DMAs
                wvraw = fp.tile([4, HD], f32, tag="wvraw")
                for h in range(4):
                    nc.sync.dma_start(
                        out=wvraw[h:h + 1, :],
                        in_=acc2_sb[h:h + 1, h * HD:(h + 1) * HD],
                    )
                seps = fp.tile([4, 1], f32, tag="seps")
                nc.vector.tensor_scalar(
                    out=seps[:], in0=s_sb[:], scalar1=1e-6, scalar2=None, op0=Alu.add,
                )
                rec = fp.tile([4, 1], f32, tag="rec")
                nc.vector.reciprocal(out=rec[:], in_=seps[:])
                wv = fp.tile([4, HD], f32, tag="wv")
                nc.vector.tensor_scalar(
                    out=wv[:], in0=wvraw[:], scalar1=rec[:, :1], scalar2=None,
                    op0=Alu.mult,
                )
                # RMS over all 512 = 4 partitions x 128
                sq = fp.tile([4, HD], f32, tag="sq")
                nc.vector.tensor_tensor(out=sq[:], in0=wv[:], in1=wv[:], op=Alu.mult)
                sqs = fp.tile([4, 1], f32, tag="sqs")
                nc.vector.tensor_reduce(out=sqs[:], in_=sq[:], axis=X, op=Alu.add)
                rmsp = tinyp.tile([4, 1], f32, tag="tiny")
                nc.tensor.matmul(
                    out=rmsp[0:1, 0:1], lhsT=sqs[:], rhs=ones4c_t[:],
                    start=True, stop=True,
                )
                msq = fp.tile([1, 1], f32, tag="msq")
                nc.vector.tensor_scalar(
                    out=msq[:], in0=rmsp[0:1, 0:1], scalar1=1.0 / D, scalar2=EPS_RMS,
                    op0=Alu.mult, op1=Alu.add,
                )
                sqr = fp.tile([1, 1], f32, tag="sqr")
                nc.scalar.activation(out=sqr[:], in_=msq[:], func=Act.Sqrt)
                rinv = fp.tile([1, 1], f32, tag="rinv")
                nc.vector.reciprocal(out=rinv[:], in_=sqr[:])
                r4p = tinyp.tile([4, 1], f32, tag="tiny")
                nc.tensor.matmul(
                    out=r4p[:], lhsT=ones4r_t[:], rhs=rinv[:], start=True, stop=True
                )
                r4_sb = fp.tile([4, 1], f32, tag="r4_sb")
                nc.vector.tensor_copy(out=r4_sb[:], in_=r4p[:])
                wvn = fp.tile([4, HD], f32, tag="wvn")
                nc.vector.tensor_scalar(
                    out=wvn[:], in0=wv[:], scalar1=r4_sb[:, :1], scalar2=None,
                    op0=Alu.mult,
                )
                wvf = fp.tile([4, HD], f32, tag="wvf")
                nc.vector.tensor_tensor(out=wvf[:], in0=wvn[:], in1=rmsr_t[:], op=Alu.mult)
                # gate head: u = sigmoid(wv . gate_W + gate_b) * (S > 0)
                # sigmoid via the Exp table (avoids an activation-table switch)
                gwm = fp.tile([4, HD], f32, tag="gwm")
                nc.vector.tensor_tensor(out=gwm[:], in0=wv[:], in1=gwr_t[:], op=Alu.mult)
                gl = fp.tile([4, 1], f32, tag="gl")
                nc.vector.tensor_reduce(out=gl[:], in_=gwm[:], axis=X, op=Alu.add)
                # en = exp(-(gl + gb)) via the Exp table: bias = -gb, scale = -1
                en = fp.tile([4, 1], f32, tag="en")
                nc.scalar.activation(
                    out=en[:], in_=gl[:], func=Act.Exp,
                    bias=ngb4_t[:, :1], scale=-1.0,
                )
                den = fp.tile([4, 1], f32, tag="den")
                nc.vector.tensor_scalar(
                    out=den[:], in0=en[:], scalar1=1.0, scalar2=None, op0=Alu.add,
                )
                sg = fp.tile([4, 1], f32, tag="sg")
                nc.vector.reciprocal(out=sg[:], in_=den[:])
                valid = fp.tile([4, 1], f32, tag="valid")
                nc.vector.tensor_scalar(
                    out=valid[:], in0=s_sb[:], scalar1=0.0, scalar2=None, op0=Alu.is_gt,
                )
                u = fp.tile([4, 1], f32, tag="u")
                nc.vector.tensor_tensor(out=u[:], in0=sg[:], in1=valid[:], op=Alu.mult)
                ue = fp.tile([4, HD], f32, tag="ue")
                nc.vector.tensor_scalar(
                    out=ue[:], in0=wvf[:], scalar1=0.0, scalar2=u[:, :1],
                    op0=Alu.mult, op1=Alu.add,
                )
                nc.sync.dma_start(out=out_d[b, 0:4, :], in_=wvf[:])
                nc.sync.dma_start(out=out_d[b, 4:8, :], in_=ue[:])

    nc.finalize()
    _NC_CACHE["nc"] = nc
    return nc


def _host_prep(inputs):
    tokens_w = np.asarray(inputs["tokens_w"], dtype=np.int32)
    prev_ids = np.asarray(inputs["prev_ids_overlap"], dtype=np.int32)
    mask_bool = np.asarray(inputs["mask_bool"])
    embed_table = np.asarray(inputs["embed_table"], dtype=np.float32)
    engram_table = np.asarray(inputs["engram_table"], dtype=np.float32)
    gate_logit = np.asarray(inputs["gate_logit"], dtype=np.float32)
    temp = np.asarray(inputs["temp"], dtype=np.float32)
    sal_W = np.asarray(inputs["sal_W"], dtype=np.float32)
    sal_b = np.asarray(inputs["sal_b"], dtype=np.float32)
    gate_W = np.asarray(inputs["gate_W"], dtype=np.float32)
    gate_b = np.asarray(inputs["gate_b"], dtype=np.float32)
    rms_scale = np.asarray(inputs["rms_scale"], dtype=np.float32)

    # ---- hashed n-gram lookup (uint32 rolling hash, as in reference) ----
    cur = np.where(tokens_w == 0, 0, tokens_w)
    prv = np.where(prev_ids == 0, 0, prev_ids)
    full_seq = np.concatenate([prv, cur], axis=1).astype(np.uint32)  # (B, O+T)
    primes = _engram_primes()                                        # (H, NG)
    hash_sums = np.zeros((B, T, H), dtype=np.uint32)
    for i in range(NG):
        chunk = full_seq[:, O - i:O + T - i]                         # (B, T)
        hash_sums += chunk[:, :, None] * primes[None, None, :, i]
    lookup = (hash_sums % np.uint32(M)).astype(np.int64)             # (B, T, H)
    rows = (lookup * H + np.arange(H)[None, None, :]).astype(np.int32)

    # ---- param folding ----
    gate = 1.0 / (1.0 + np.exp(-gate_logit.astype(np.float64)))      # (H, HD)
    tf = np.log1p(np.exp(temp.astype(np.float64))) + 0.3             # (H,)
    gate32 = gate.astype(np.float32)
    gated = engram_table * gate32[None, :, :]                        # (M, H, HD)
    SWt = (sal_W.astype(np.float64) / tf[None, :]).astype(np.float32)  # (D, 4)
    SWt_r = SWt.reshape(H, HD, 4)
    aug = np.empty((M * H, ELEM), dtype=np.float32)
    aug[:, :HD] = gated.reshape(M * H, HD)
    for h in range(H):
        aug[h::H, HD:] = gated[:, h, :] @ SWt_r[h]
    EWb = ((embed_table.astype(np.float64) @ sal_W.astype(np.float64)
            + sal_b[None, :].astype(np.float64)) / tf[None, :]).astype(np.float32)

    # ---- mask compaction: pack valid tokens, NTC tiles of P per batch ----
    EW_full = EWb[tokens_w]                                          # (B, T, 4)
    tok_c = np.zeros((B, CP), dtype=np.float32)
    ew_c = np.full((B, CP, 4), PAD_LOGIT, dtype=np.float32)
    idx_c = np.zeros((B, CP, 4), dtype=np.int32)
    for b in range(B):
        v = np.nonzero(mask_bool[b])[0]
        nv = len(v)
        assert nv <= CP, f"batch {b}: {nv} valid tokens > {CP} slots"
        tok_c[b, :nv] = tokens_w[b, v]
        ew_c[b, :nv] = EW_full[b, v]
        idx_c[b, :nv] = rows[b, v]

    # ---- per-core input layout: [p, (b, tile)] with p = slot % 128 ----
    tok_pt = np.ascontiguousarray(
        tok_c.reshape(B, NTC, P).transpose(2, 0, 1).reshape(P, B * NTC)
    )
    idx_pt = np.ascontiguousarray(
        idx_c.reshape(B, NTC, P, 4).transpose(2, 0, 1, 3).reshape(P, B * NTC * 4)
    )
    ew_pt = np.ascontiguousarray(
        ew_c.reshape(B, NTC, P, 4).transpose(2, 0, 1, 3).reshape(P, B * NTC * 4)
    )

    iota_f = np.ascontiguousarray(
        np.broadcast_to(np.arange(P, dtype=np.float32), (P, P))
    )
    gwr = np.ascontiguousarray(
        np.broadcast_to(gate_W[:, 0][None, :], (4, HD))
    ).astype(np.float32)
    rmsr = rms_scale.reshape(4, HD).copy()
    ngb4 = np.full((4, 1), -float(gate_b[0]), dtype=np.float32)

    shared = {
        "aug": aug, "emb": embed_table, "iotaf": iota_f,
        "ident4": np.eye(4, dtype=np.float32),
        "gwr": gwr, "rmsr": rmsr, "ngb4": ngb4,
        "onesc": np.ones((P, 1), dtype=np.float32),
        "ones4c": np.ones((4, 1), dtype=np.float32),
        "ones4r": np.ones((1, 4), dtype=np.float32),
    }
    in_maps = []
    for k in range(NCORES):
        cs, ce = k * BL * NTC, (k + 1) * BL * NTC
        m = dict(shared)
        m["idx4"] = np.ascontiguousarray(idx_pt[:, cs * 4:ce * 4])
        m["ewt"] = np.ascontiguousarray(ew_pt[:, cs * 4:ce * 4])
        m["tokf"] = np.ascontiguousarray(tok_pt[:, cs:ce])
        in_maps.append(m)
    return in_maps


def _run(inputs, trace=False, **kw):
    from concourse.bass_utils import run_bass_kernel_spmd

    nc = _build_nc()
    in_maps = _host_prep(inputs)
    r = run_bass_kernel_spmd(
        nc, in_maps, list(range(NCORES)), trace=trace, **kw
    )
    out = np.concatenate([r.results[k]["out"].reshape(BL, 2 * D)
                          for k in range(NCORES)], axis=0)
    return out, r


def kernel(**inputs):
    out, _ = _run(inputs, trace=False)
    return out
